# revision 13
# baseline (speedup 1.0000x reference)
"""BaseCrossAttention Trainium2 kernel.

Full inputs -> full output. Shards batch B=32 across 8 NeuronCores (4 each),
builds one SPMD Bass/Tile program, runs via run_bass_kernel_spmd.

Math notes (exact rewrites of the reference):
 - softmax over N is shift-invariant, so the uniform (1-tcond)*-1e6 mask term
   cancels; gating is fully carried by v *= tcond.
 - LayerNorm(y) is scale-invariant per row, and y only feeds LayerNorm, so the
   softmax denominator (uniform over d) cancels: we use p = exp(logits)
   unnormalized and never divide.
Layouts: attention runs "transposed" (feature dim on partitions, tokens on the
free axis) so all contractions land on the partition axis naturally; the three
layout flips (ln(x), ln(xf), h) are 128x128 bf16 DMA XBAR transposes.
"""

import sys

sys.path.insert(0, "/opt/trn_rl_repo")

import numpy as np

import concourse.bass as bass
import concourse.tile as tile
from concourse import bacc, mybir
from concourse.bass_utils import run_bass_kernel_spmd

F32 = mybir.dt.float32
BF16 = mybir.dt.bfloat16

B, T, D = 32, 1024, 1024
N, L = 256, 768
H = 16
HD = 64
TE = 2048
NCORES = 8
BPC = B // NCORES  # 4 batches per core
P = 128
TC = T // P   # 8 t-chunks
DC = D // P   # 8 d-chunks
LC = L // P   # 6 l-chunks
NC2 = N // P  # 2 n-chunks
TH = T // 512  # 2 t-halves (PSUM free limit)

LAST_RESULT = None
LAST_NC = None


def _ln_stats(nc, pool, xc, fd, sub, eps_c):
    """bn_stats/aggr over free dim -> (neg_mean_times_r, rstd) [128,1] f32."""
    nsub = fd // sub
    stats = pool.tile([P, nsub, 6], F32, tag="stats")
    for i in range(nsub):
        nc.vector.bn_stats(out=stats[:, i, :], in_=xc[:, i * sub:(i + 1) * sub])
    mv = pool.tile([P, 2], F32, tag="mv")
    nc.vector.bn_aggr(out=mv[:], in_=stats[:])
    sd = pool.tile([P, 1], F32, tag="sd")
    nc.scalar.activation(out=sd, in_=mv[:, 1:2],
                         func=mybir.ActivationFunctionType.Sqrt,
                         bias=eps_c, scale=1.0)
    rt = pool.tile([P, 1], F32, tag="rt")
    nc.vector.reciprocal(out=rt, in_=sd)
    nmr = pool.tile([P, 1], F32, tag="nmr")
    # nmr = -mean * rstd
    nc.vector.tensor_scalar(out=nmr, in0=mv[:, 0:1], scalar1=rt, scalar2=-1.0,
                            op0=mybir.AluOpType.mult, op1=mybir.AluOpType.mult)
    return nmr, rt


def build(nc, flags):
    """Emit the whole per-core program under a TileContext."""
    d = {}
    d["x"] = nc.dram_tensor("x", [BPC, T, D], F32, kind="ExternalInput").ap()
    d["xf"] = nc.dram_tensor("xf", [BPC, N, L], F32, kind="ExternalInput").ap()
    d["emb"] = nc.dram_tensor("emb", [BPC, TE], F32, kind="ExternalInput").ap()
    d["gate"] = nc.dram_tensor("gate", [BPC, 1], F32, kind="ExternalInput").ap()
    for nm, shp in [("Wq", [D, D]), ("Wk", [L, D]), ("Wv", [L, D]),
                    ("We", [TE, 2 * D]), ("Wo", [D, D])]:
        d[nm] = nc.dram_tensor(nm, shp, F32, kind="ExternalInput").ap()
    for nm, n in [("bq", D), ("bk", D), ("bv", D), ("be", 2 * D), ("bo", D),
                  ("ln_x_g", D), ("ln_x_b", D), ("ln_t_g", L), ("ln_t_b", L),
                  ("ln_y_g", D), ("ln_y_b", D)]:
        d[nm] = nc.dram_tensor(nm, [n], F32, kind="ExternalInput").ap()
    out = nc.dram_tensor("out", [BPC, T, D], F32, kind="ExternalOutput").ap()

    from contextlib import ExitStack

    with tile.TileContext(nc) as tc, ExitStack() as ctx:
        consts = ctx.enter_context(tc.tile_pool(name="consts", bufs=1))
        wpool = ctx.enter_context(tc.tile_pool(name="wpool", bufs=1))
        big = ctx.enter_context(tc.tile_pool(name="big", bufs=3))
        med = ctx.enter_context(tc.tile_pool(name="med", bufs=2))
        small = ctx.enter_context(tc.tile_pool(name="small", bufs=3))
        stat1 = ctx.enter_context(tc.tile_pool(name="stat1", bufs=1))
        ps = ctx.enter_context(tc.tile_pool(name="ps", bufs=8, space="PSUM"))

        # ---------- constants / weight prep ----------
        ones_bf = consts.tile([P, P], BF16)
        nc.vector.memset(ones_bf, 1.0)
        eps_c = consts.tile([P, 1], F32)
        nc.vector.memset(eps_c, 1e-5)

        def vec_pc(name, nchunk, dtype=F32):
            # DRAM (nchunk*128,) -> SBUF [128, nchunk] (d-major chunking)
            t = consts.tile([P, nchunk], dtype, tag="v_" + name)
            nc.sync.dma_start(out=t[:], in_=d[name].rearrange("(c p) -> p c", p=P))
            return t

        bq_t = vec_pc("bq", DC)
        bk_t = vec_pc("bk", DC)
        bo_t = vec_pc("bo", DC)
        be_t = vec_pc("be", 2 * DC)
        gy_t = vec_pc("ln_y_g", DC)
        by_t = vec_pc("ln_y_b", DC)
        gx_t = vec_pc("ln_x_g", DC)
        gt_t = vec_pc("ln_t_g", LC)
        if flags["bx"]:
            bx_t = vec_pc("ln_x_b", DC)
        if flags["bt"]:
            bt_t = vec_pc("ln_t_b", LC)
        if flags["bv"]:
            bv_bc = consts.tile([P, D], F32)
            nc.sync.dma_start(
                out=bv_bc[:],
                in_=bass.AP(tensor=d["bv"].tensor, offset=d["bv"].offset,
                            ap=[[0, P]] + list(d["bv"].ap)))

        gate_t = []
        for b in range(BPC):
            gt = consts.tile([P, 1], F32, tag=f"gate{b}")
            gb = d["gate"][b:b + 1, :]
            nc.sync.dma_start(
                out=gt[:],
                in_=bass.AP(tensor=gb.tensor, offset=gb.offset,
                            ap=[[0, P], [1, 1]]))
            gate_t.append(gt)

        # weights -> bf16 resident, LN gains folded into Wq/Wk/Wv rows unless
        # a tile-affine pass is needed (nonzero LN bias).
        def load_w(name, kchunks, width, scale_t):
            wb = wpool.tile([P, kchunks, width], BF16, tag="W" + name)
            for k in range(kchunks):
                st = med.tile([P, width], F32, tag="wstage")
                nc.sync.dma_start(out=st[:], in_=d[name][k * P:(k + 1) * P, :])
                if scale_t is None:
                    nc.vector.tensor_copy(out=wb[:, k, :], in_=st[:])
                else:
                    nc.vector.tensor_scalar_mul(
                        out=wb[:, k, :], in0=st[:], scalar1=scale_t[:, k:k + 1])
            return wb

        Wq_bf = load_w("Wq", DC, D, None if flags["bx"] else gx_t)
        Wk_bf = load_w("Wk", LC, D, None if flags["bt"] else gt_t)
        Wv_bf = load_w("Wv", LC, D, None if flags["bt"] else gt_t)
        Wo_bf = load_w("Wo", DC, D, None)

        # ---------- emb branch: emb_outT[d2, b] = We.T @ silu(emb).T ----------
        embT = consts.tile([P, TE // P, BPC], F32)
        for b in range(BPC):
            nc.sync.dma_start(
                out=embT[:, :, b],
                in_=d["emb"][b, :].rearrange("(c p) -> p c", p=P))
        embsT = consts.tile([P, TE // P, BPC], BF16)
        embsg = consts.tile([P, TE // P, BPC], F32)
        nc.scalar.activation(out=embsg[:], in_=embT[:],
                             func=mybir.ActivationFunctionType.Sigmoid)
        nc.vector.tensor_mul(out=embsT[:], in0=embT[:], in1=embsg[:])
        emb_outT = consts.tile([P, 2 * DC, BPC], F32)
        for mg in range(2):
            psl = [ps.tile([P, 512], F32, tag="ps", name=f"psemb{mg}_{i}")
                   for i in range(8)]
            for k in range(TE // P):
                st = med.tile([P, DC * P], F32, tag="wstage")
                nc.sync.dma_start(
                    out=st[:], in_=d["We"][k * P:(k + 1) * P,
                                           mg * 1024:(mg + 1) * 1024])
                stb = med.tile([P, DC * P], BF16, tag="wstage_bf")
                nc.vector.tensor_copy(out=stb[:], in_=st[:])
                for m8 in range(8):
                    nc.tensor.matmul(psl[m8][:, :BPC],
                                     lhsT=stb[:, m8 * P:(m8 + 1) * P],
                                     rhs=embsT[:, k, :],
                                     start=(k == 0), stop=(k == TE // P - 1))
            for m8 in range(8):
                mc = mg * 8 + m8
                nc.scalar.activation(out=emb_outT[:, mc, :], in_=psl[m8][:, :BPC],
                                     func=mybir.ActivationFunctionType.Identity,
                                     bias=be_t[:, mc:mc + 1])
        # scale half becomes (1 + scale)
        nc.vector.tensor_scalar_add(out=emb_outT[:, 0:DC, :],
                                    in0=emb_outT[:, 0:DC, :], scalar1=1.0)

        # ---------- per batch ----------
        for b in range(BPC):
            # ---- LN(x) -> lnx bf16 [t,d], then transpose -> lnxT [d,t]
            lnx = big.tile([P, TC, D], BF16, tag="big")
            for t in range(TC):
                xc = small.tile([P, D], F32, tag="xc")
                nc.sync.dma_start(out=xc[:], in_=d["x"][b, t * P:(t + 1) * P, :])
                nmr, rt = _ln_stats(nc, small, xc, D, 512, eps_c)
                nc.scalar.activation(out=lnx[:, t, :], in_=xc[:],
                                     func=mybir.ActivationFunctionType.Identity,
                                     bias=nmr, scale=rt)
            lnxT = big.tile([P, DC, T], BF16, tag="big")
            for t in range(TC):
                for dc in range(DC):
                    nc.sync.dma_start_transpose(
                        out=lnxT[:, dc, t * P:(t + 1) * P],
                        in_=lnx[:, t, dc * P:(dc + 1) * P])
            if flags["bx"]:
                for dc in range(DC):
                    nc.vector.tensor_scalar(
                        out=lnxT[:, dc, :], in0=lnxT[:, dc, :],
                        scalar1=gx_t[:, dc:dc + 1], scalar2=bx_t[:, dc:dc + 1],
                        op0=mybir.AluOpType.mult, op1=mybir.AluOpType.add)

            # ---- LN(xf) -> xfn bf16 [n,l] -> xfnT [l,n]
            xfn = med.tile([P, NC2, L], BF16, tag="xfn")
            for n in range(NC2):
                fc = small.tile([P, L], F32, tag="xc")
                nc.sync.dma_start(out=fc[:], in_=d["xf"][b, n * P:(n + 1) * P, :])
                nmr, rt = _ln_stats(nc, small, fc, L, 256, eps_c)
                nc.scalar.activation(out=xfn[:, n, :], in_=fc[:],
                                     func=mybir.ActivationFunctionType.Identity,
                                     bias=nmr, scale=rt)
            xfnT = med.tile([P, LC, N], BF16, tag="xfnT")
            for n in range(NC2):
                for lc in range(LC):
                    nc.sync.dma_start_transpose(
                        out=xfnT[:, lc, n * P:(n + 1) * P],
                        in_=xfn[:, n, lc * P:(lc + 1) * P])
            if flags["bt"]:
                for lc in range(LC):
                    nc.vector.tensor_scalar(
                        out=xfnT[:, lc, :], in0=xfnT[:, lc, :],
                        scalar1=gt_t[:, lc:lc + 1], scalar2=bt_t[:, lc:lc + 1],
                        op0=mybir.AluOpType.mult, op1=mybir.AluOpType.add)

            # ---- kT[d,n] = Wk.T @ xfnT ; v[n,d] = xfnT.T @ Wv (gated)
            kT = med.tile([P, DC, N], BF16, tag="kT")
            for m in range(DC):
                pk = ps.tile([P, 512], F32, tag="ps")
                for lc in range(LC):
                    nc.tensor.matmul(pk[:, :N], lhsT=Wk_bf[:, lc, m * P:(m + 1) * P],
                                     rhs=xfnT[:, lc, :],
                                     start=(lc == 0), stop=(lc == LC - 1))
                nc.vector.tensor_scalar_add(out=kT[:, m, :], in0=pk[:, :N],
                                            scalar1=bk_t[:, m:m + 1])
            v_bf = med.tile([P, NC2, D], BF16, tag="v")
            for n in range(NC2):
                for dh in range(2):
                    pv = ps.tile([P, 512], F32, tag="ps")
                    for lc in range(LC):
                        nc.tensor.matmul(pv[:], lhsT=xfnT[:, lc, n * P:(n + 1) * P],
                                         rhs=Wv_bf[:, lc, dh * 512:(dh + 1) * 512],
                                         start=(lc == 0), stop=(lc == LC - 1))
                    if flags["bv"]:
                        nc.vector.tensor_tensor(
                            out=pv[:], in0=pv[:],
                            in1=bv_bc[:, dh * 512:(dh + 1) * 512],
                            op=mybir.AluOpType.add)
                    nc.vector.tensor_scalar_mul(
                        out=v_bf[:, n, dh * 512:(dh + 1) * 512], in0=pv[:],
                        scalar1=gate_t[b])

            # ---- qT[d,t] = Wq.T @ lnxT  (+bq)
            qT = big.tile([P, DC, T], BF16, tag="big")
            for m in range(DC):
                for th in range(TH):
                    pq = ps.tile([P, 512], F32, tag="ps")
                    for k in range(DC):
                        nc.tensor.matmul(pq[:], lhsT=Wq_bf[:, k, m * P:(m + 1) * P],
                                         rhs=lnxT[:, k, th * 512:(th + 1) * 512],
                                         start=(k == 0), stop=(k == DC - 1))
                    nc.vector.tensor_scalar_add(
                        out=qT[:, m, th * 512:(th + 1) * 512], in0=pq[:],
                        scalar1=bq_t[:, m:m + 1])

            # ---- attention: per head-pair hp (d-chunk = heads 2hp, 2hp+1)
            yT = big.tile([P, DC, T], BF16, tag="big")
            for hp in range(DC):
                pT = [med.tile([P, NC2, T], BF16, tag="pT", name=f"pT{hp}_{j}")
                      for j in range(2)]
                for j in range(2):
                    r0, r1 = 64 * j, 64 * j + 64
                    for n in range(NC2):
                        for th in range(TH):
                            pa = ps.tile([P, 512], F32, tag="ps")
                            nc.tensor.matmul(
                                pa[:], lhsT=kT[r0:r1, hp, n * P:(n + 1) * P],
                                rhs=qT[r0:r1, hp, th * 512:(th + 1) * 512],
                                start=True, stop=True)
                            nc.scalar.activation(
                                out=pT[j][:, n, th * 512:(th + 1) * 512],
                                in_=pa[:], func=mybir.ActivationFunctionType.Exp)
                # softmax denominators s_h[t] = sum_n p (row-replicated via
                # ones-matmul); reciprocal into the head's 64-row band
                rs_pair = med.tile([P, T], BF16, tag="rs")
                for j in range(2):
                    for th in range(TH):
                        ps_s = ps.tile([P, 512], F32, tag="ps")
                        for n in range(NC2):
                            nc.tensor.matmul(
                                ps_s[:], lhsT=ones_bf[:],
                                rhs=pT[j][:, n, th * 512:(th + 1) * 512],
                                start=(n == 0), stop=(n == NC2 - 1))
                        with nc.allow_low_precision(reason="softmax denom bf16"):
                            nc.vector.reciprocal(
                                out=rs_pair[64 * j:64 * j + 64,
                                            th * 512:(th + 1) * 512],
                                in_=ps_s[64 * j:64 * j + 64, :])
                for th in range(TH):
                    py = ps.tile([P, 512], F32, tag="ps")
                    for j in range(2):
                        h = 2 * hp + j
                        for n in range(NC2):
                            nc.tensor.matmul(
                                py[64 * j:64 * j + 64, :],
                                lhsT=v_bf[:, n, h * HD:(h + 1) * HD],
                                rhs=pT[j][:, n, th * 512:(th + 1) * 512],
                                start=(n == 0), stop=(n == NC2 - 1),
                                tile_position=(0, 64 * j))
                    nc.vector.tensor_tensor(
                        out=yT[:, hp, th * 512:(th + 1) * 512], in0=py[:],
                        in1=rs_pair[:, th * 512:(th + 1) * 512],
                        op=mybir.AluOpType.mult)

            # ---- LN(y) over d (partition axis) via PE column sums
            sums = stat1.tile([P, T], F32, tag="sums")
            sqs = stat1.tile([P, T], F32, tag="sqs")
            for th in range(TH):
                pss = ps.tile([P, 512], F32, tag="ps")
                psq = ps.tile([P, 512], F32, tag="ps")
                for dc in range(DC):
                    y2h = small.tile([P, 512], BF16, tag="y2h")
                    nc.vector.tensor_mul(out=y2h[:],
                                         in0=yT[:, dc, th * 512:(th + 1) * 512],
                                         in1=yT[:, dc, th * 512:(th + 1) * 512])
                    nc.tensor.matmul(pss[:], lhsT=ones_bf[:],
                                     rhs=yT[:, dc, th * 512:(th + 1) * 512],
                                     start=(dc == 0), stop=(dc == DC - 1))
                    nc.tensor.matmul(psq[:], lhsT=ones_bf[:], rhs=y2h[:],
                                     start=(dc == 0), stop=(dc == DC - 1))
                nc.vector.tensor_copy(out=sums[:, th * 512:(th + 1) * 512], in_=pss[:])
                nc.vector.tensor_copy(out=sqs[:, th * 512:(th + 1) * 512], in_=psq[:])
            # mean in-place on sums, var in-place on sqs
            nc.vector.tensor_scalar_mul(out=sums[:], in0=sums[:], scalar1=1.0 / D)
            nc.vector.tensor_scalar_mul(out=sqs[:], in0=sqs[:], scalar1=1.0 / D)
            m2 = stat1.tile([P, T], F32, tag="m2")
            nc.vector.tensor_mul(out=m2[:], in0=sums[:], in1=sums[:])
            nc.vector.tensor_sub(out=sqs[:], in0=sqs[:], in1=m2[:])
            nc.scalar.activation(out=m2[:], in_=sqs[:],
                                 func=mybir.ActivationFunctionType.Sqrt,
                                 bias=eps_c, scale=1.0)
            r_bf = stat1.tile([P, T], BF16, tag="r_bf")
            mr_bf = stat1.tile([P, T], BF16, tag="mr_bf")
            with nc.allow_low_precision(reason="rstd consumed by bf16 pipeline"):
                nc.vector.reciprocal(out=r_bf[:], in_=m2[:])
                nc.vector.tensor_mul(out=mr_bf[:], in0=sums[:], in1=r_bf[:])

            # stylization affine per d-chunk: alpha = gy*(1+s), beta = by*(1+s)+sh
            alpha = stat1.tile([P, DC], F32, tag="alpha")
            beta = stat1.tile([P, DC], F32, tag="beta")
            nc.vector.tensor_mul(out=alpha[:], in0=gy_t[:], in1=emb_outT[:, 0:DC, b])
            nc.vector.tensor_mul(out=beta[:], in0=by_t[:], in1=emb_outT[:, 0:DC, b])
            nc.vector.tensor_add(out=beta[:], in0=beta[:],
                                 in1=emb_outT[:, DC:2 * DC, b])

            # ---- z = (yT - mean)*rstd ; siluT = Silu(z*alpha + beta)
            siluT = big.tile([P, DC, T], BF16, tag="big")
            for dc in range(DC):
                zc = small.tile([P, T], BF16, tag="zc")
                nc.vector.tensor_mul(out=zc[:], in0=yT[:, dc, :], in1=r_bf[:])
                nc.vector.tensor_sub(out=zc[:], in0=zc[:], in1=mr_bf[:])
                nc.vector.tensor_scalar(out=zc[:], in0=zc[:],
                                        scalar1=alpha[:, dc:dc + 1],
                                        scalar2=beta[:, dc:dc + 1],
                                        op0=mybir.AluOpType.mult,
                                        op1=mybir.AluOpType.add)
                uc = med.tile([P, T], BF16, tag="uc")
                nc.scalar.activation(out=uc[:], in_=zc[:],
                                     func=mybir.ActivationFunctionType.Sigmoid)
                nc.vector.tensor_mul(out=siluT[:, dc, :], in0=zc[:], in1=uc[:])

            # ---- hT = Wo.T @ siluT (+bo), transpose back, out = x + h
            hT = big.tile([P, DC, T], BF16, tag="big")
            for m in range(DC):
                for th in range(TH):
                    ph = ps.tile([P, 512], F32, tag="ps")
                    for k in range(DC):
                        nc.tensor.matmul(ph[:], lhsT=Wo_bf[:, k, m * P:(m + 1) * P],
                                         rhs=siluT[:, k, th * 512:(th + 1) * 512],
                                         start=(k == 0), stop=(k == DC - 1))
                    nc.scalar.activation(out=hT[:, m, th * 512:(th + 1) * 512],
                                         in_=ph[:],
                                         func=mybir.ActivationFunctionType.Identity,
                                         bias=bo_t[:, m:m + 1])
            h_nat = big.tile([P, TC, D], BF16, tag="big")
            for dc in range(DC):
                for t in range(TC):
                    nc.sync.dma_start_transpose(
                        out=h_nat[:, t, dc * P:(dc + 1) * P],
                        in_=hT[:, dc, t * P:(t + 1) * P])
            for t in range(TC):
                xc2 = small.tile([P, D], F32, tag="xc")
                nc.sync.dma_start(out=xc2[:], in_=d["x"][b, t * P:(t + 1) * P, :])
                oc = med.tile([P, D], F32, tag="oc")
                nc.vector.tensor_add(out=oc[:], in0=xc2[:], in1=h_nat[:, t, :])
                nc.sync.dma_start(out=out[b, t * P:(t + 1) * P, :], in_=oc[:])
    return nc


def kernel(**inputs):
    global LAST_RESULT, LAST_NC
    x = np.asarray(inputs["x"], dtype=np.float32)
    xf = np.asarray(inputs["xf"], dtype=np.float32)
    emb = np.asarray(inputs["emb"], dtype=np.float32)
    cond = np.asarray(inputs["cond_type"]).reshape(B).astype(np.int64)
    gate = ((cond % 10) > 0).astype(np.float32).reshape(B, 1)

    wnames = ["Wq", "Wk", "Wv", "We", "Wo", "bq", "bk", "bv", "be", "bo",
              "ln_x_g", "ln_x_b", "ln_t_g", "ln_t_b", "ln_y_g", "ln_y_b"]
    w = {n: np.ascontiguousarray(np.asarray(inputs[n], dtype=np.float32))
         for n in wnames}

    flags = {
        "bx": bool(np.any(w["ln_x_b"] != 0.0)),
        "bt": bool(np.any(w["ln_t_b"] != 0.0)),
        "bv": bool(np.any(w["bv"] != 0.0)),
    }

    nc = bacc.Bacc("TRN2", target_bir_lowering=False, debug=False,
                   enable_asserts=False, num_devices=NCORES)
    build(nc, flags)
    nc.compile()

    in_maps = []
    for i in range(NCORES):
        s = slice(i * BPC, (i + 1) * BPC)
        m = {"x": np.ascontiguousarray(x[s]),
             "xf": np.ascontiguousarray(xf[s]),
             "emb": np.ascontiguousarray(emb[s]),
             "gate": np.ascontiguousarray(gate[s])}
        m.update(w)
        in_maps.append(m)

    LAST_NC = nc
    res = run_bass_kernel_spmd(nc, in_maps, core_ids=list(range(NCORES)))
    LAST_RESULT = res
    return np.concatenate([r["out"] for r in res.results], axis=0)
